# revision 8
# baseline (speedup 1.0000x reference)
"""Trainium2 Bass kernel for the conv1d-stack + MLP ragged-sequence model.

Strategy:
  - Pure data parallelism over 8 NeuronCores: 8 samples per core.
  - Samples are sorted by valid length (descending) and dealt round-robin to
    cores, so every core runs an IDENTICAL (SPMD) program whose per-slot
    sizes are the per-rank-group maximum length (exploits raggedness).
  - L1 (52% of all matmul columns) runs in fp8-e4m3 with DoubleRow perf
    mode (2 cols/cycle, 2 k-tiles per matmul).  Accuracy is held by a 3-term
    hi/lo decomposition: x_h*w_h + x_h*w_l + x_l*w_h (the dropped lo*lo term
    is ~0.4% of each product).  The host ships x as three fp8 slabs
    [x_h | x_l | x_h<<1]; slab adjacency gives every DoubleRow pairing a
    legal strided AP view.  10 polyphase taps * 3 products = 15 k-tiles ->
    8 DoubleRow matmuls = 4N column-cycles vs 5N for fp32r.
  - L2-L5 stay fp32r matmuls (phase-split stride-1 reads, 1 col/cycle).
  - L6, L7 and small-slot L4 run in bf16: fp32r moving operands under 256
    columns drop to 1/4 rate on a warm PE, bf16 keeps 1 col/cycle.
  - Layers 5-7 interleave all 8 slots (column = t*8 + s) so their matmul
    reads are single contiguous slabs.
  - Slots are emitted in pairs so PSUM-eviction latency of slot j hides
    behind slot j+1's matmuls; the slot 4-7 tail (L5/pool) is emitted in
    the middle of the remaining conv pairs to shorten the serial tail.
  - All f32 weights/biases/masks ship as ONE packed DRAM tensor (single
    DMA issue), fp8 and bf16 weights as one pack each: startup is gated by
    two small x-slab DMAs instead of a dozen serialized issues.
  - The ragged masked-max uses a host-built additive -1e30 mask (a data
    input, SPMD-safe), then the 3-layer MLP runs on-device.
"""

import os
import sys

for _p in ("/opt/trn_rl_repo",):
    if _p not in sys.path and os.path.isdir(_p):
        sys.path.insert(0, _p)

import numpy as np
import ml_dtypes

import concourse.bass as bass  # noqa: F401  (registers types)
from concourse import bacc
import concourse.tile as tile
import concourse.mybir as mybir
from concourse.bass_utils import run_bass_kernel_spmd

F32 = mybir.dt.float32
F32R = mybir.dt.float32r
F8 = mybir.dt.float8e4
BF16 = mybir.dt.bfloat16
E4NP = ml_dtypes.float8_e4m3
BFNP = ml_dtypes.bfloat16
AF = mybir.ActivationFunctionType
AX = mybir.AxisListType
DR = mybir.MatmulPerfMode.DoubleRow

N_CORES = 8
N_SLOTS = 8
B = 64
C_IN = 40
T_FULL = 8192
NEG = -1.0e30
BF4_THRESH = 256  # L4 slots with fewer moving cols than this run in bf16


def _chain(t0):
    """Per-layer valid/capacity length chain (mirrors the reference)."""
    t1 = (t0 - 10) // 2 + 1
    t2 = (t1 - 5) // 2 + 1
    t3 = (t2 - 5) // 2 + 1
    t4 = (t3 - 5) // 2 + 1
    t4p = t4 // 2
    t5 = (t4p - 5) // 2 + 1
    t5p = t5 // 2
    t6 = (t5p - 5) // 2 + 1
    t7 = (t6 - 3) // 2 + 1
    return t1, t2, t3, t4, t4p, t5, t5p, t6, t7


def _uniform_tail(caps):
    T4P0 = _chain(caps[0])[4]
    T5u = (T4P0 - 5) // 2 + 1
    T5pu = T5u // 2
    T6u = (T5pu - 5) // 2 + 1
    T7u = (T6u - 3) // 2 + 1
    return T4P0, T5u, T5pu, T6u, T7u


def _pack_cols(caps):
    """Column layout of the packed f32 weight tensor."""
    T7u = _uniform_tail(caps)[4]
    off = {}
    c = 0
    for name, w in (("w2", 480), ("w3", 480), ("w4", 480), ("w5", 480),
                    ("lw1", 128), ("lw2", 64), ("lw3", 5), ("bias", 11),
                    ("fmask", N_SLOTS * T7u)):
        off[name] = c
        c += w
    return off, c


def _build_program(caps):
    """Emit the SPMD Bass/Tile program for the given 8 slot capacities."""
    T0 = list(caps)
    T1, T2, T3, T4, T4p = [], [], [], [], []
    for t0 in T0:
        t1, t2, t3, t4, t4p, _, _, _, _ = _chain(t0)
        T1.append(t1)
        T2.append(t2)
        T3.append(t3)
        T4.append(t4)
        T4p.append(t4p)
    T4P0, T5u, T5pu, T6u, T7u = _uniform_tail(caps)
    P4E = (T4P0 + 1) // 2
    P4O = T4P0 // 2
    P5E = (T5pu + 1) // 2
    P5O = T5pu // 2
    P6E = (T6u + 1) // 2
    P6O = T6u // 2
    poff, PCOLS = _pack_cols(caps)

    nc = bacc.Bacc("TRN2", target_bir_lowering=False, debug=False)

    xs = [
        nc.dram_tensor(f"x{j}", [80, 3 * (T0[j] // 2)], F8, kind="ExternalInput")
        for j in range(N_SLOTS)
    ]
    w1dr_t = nc.dram_tensor("w1dr", [80, 8 * 192], F8, kind="ExternalInput")
    wpk_t = nc.dram_tensor("wpk", [128, PCOLS], F32R, kind="ExternalInput")
    wbf_t = nc.dram_tensor("wbf", [96, 1344], BF16, kind="ExternalInput")
    out_t = nc.dram_tensor("out", [5, N_SLOTS], F32, kind="ExternalOutput")

    with tile.TileContext(nc) as tc:
        with (
            tc.tile_pool(name="wp", bufs=1) as wp,
            tc.tile_pool(name="xp", bufs=3) as xp,
            tc.tile_pool(name="actp", bufs=2) as ap_,
            tc.tile_pool(name="catp", bufs=1) as cp,
            tc.tile_pool(name="psp", bufs=2, space="PSUM") as pp,
        ):
            # ---- PE warm-up tile memset first so the PE can ramp while the
            # first x slabs stream in (outputs are never read) ----
            wtile = ap_.tile([80, 512], F32R, tag="wtile")
            nc.gpsimd.memset(wtile[:].bitcast(F32), 0.0)

            # ---- slot-0/1 inputs first (they gate the first matmuls) ----
            x2t = [None] * N_SLOTS
            _ldq = [nc.sync, nc.gpsimd]
            _ldn = [0]

            def emit_load(j):
                th = T0[j] // 2
                x2 = xp.tile([80, 3 * th], F8, tag="x2", name=f"x2_{j}")
                xv_s = x2[0:80, :].rearrange("p (q t) -> p q t", q=3)
                xv_d = xs[j][:, :].rearrange("p (q t) -> p q t", q=3)
                cuts = [0, 2056, th] if th > 2100 else [0, th]
                for ci in range(len(cuts) - 1):
                    lo, hi = cuts[ci], cuts[ci + 1]
                    q = _ldq[_ldn[0] % 2]
                    _ldn[0] += 1
                    q.dma_start(xv_s[:, :, lo:hi], xv_d[:, :, lo:hi])
                x2t[j] = x2

            emit_load(7)
            emit_load(6)

            # ---- resident weights / constants (3 packed DMAs) ----
            w1s = wp.tile([80, 8 * 192], F8, tag="w1dr")
            nc.scalar.dma_start(w1s[:], w1dr_t[:])
            wpk = wp.tile([128, PCOLS], F32R, tag="wpk")
            nc.scalar.dma_start(wpk[:], wpk_t[:])
            wbf = wp.tile([96, 1344], BF16, tag="wbf")
            nc.scalar.dma_start(wbf[:], wbf_t[:])

            wls = {
                l: wpk[0:96, poff[f"w{l}"] : poff[f"w{l}"] + 480]
                for l in (2, 3, 4, 5)
            }
            w4bf = wbf[0:96, 0:480]
            w6bf = wbf[0:96, 480:960]
            w7bf = wbf[0:96, 960:1344]
            lw1s = wpk[0:128, poff["lw1"] : poff["lw1"] + 128]
            lw2s = wpk[0:128, poff["lw2"] : poff["lw2"] + 64]
            lw3s = wpk[0:64, poff["lw3"] : poff["lw3"] + 5]
            bs = wpk[0:128, poff["bias"] : poff["bias"] + 11].bitcast(F32)
            fms = wpk[0:128, poff["fmask"] : poff["fmask"] + N_SLOTS * T7u].bitcast(F32)

            # ---- PE warm-up ----
            ps_w = pp.tile([96, 480], F32, tag="conv")
            for wi in range(6):
                nc.tensor.matmul(
                    ps_w[0:96, 0:480],
                    wtile[0:80, 0:96],
                    wtile[0:80, 0:480],
                    start=True,
                    stop=True,
                )

            # ---- concatenated slot-interleaved tail buffers (phase-split) ----
            p4Ecat = cp.tile([96, N_SLOTS * P4E], F32R, tag="p4Ecat")
            p4Ocat = cp.tile([96, N_SLOTS * P4O], F32R, tag="p4Ocat")
            nc.gpsimd.memset(p4Ecat[:].bitcast(F32), 0.0)
            nc.gpsimd.memset(p4Ocat[:].bitcast(F32), 0.0)
            p5Ecat = cp.tile([96, N_SLOTS * P5E], BF16, tag="p5Ecat")
            p5Ocat = cp.tile([96, N_SLOTS * P5O], BF16, tag="p5Ocat")
            b6Ecat = cp.tile([96, N_SLOTS * P6E], BF16, tag="b6Ecat")
            b6Ocat = cp.tile([96, N_SLOTS * P6O], BF16, tag="b6Ocat")
            b7cat = cp.tile([128, N_SLOTS * T7u], F32, tag="b7cat")

            def act(dst_ap, src_ap, bias_col, func=AF.Relu, prange=96, scale=1.0):
                nc.scalar.activation(
                    dst_ap, src_ap, func,
                    bias=bs[0:prange, bias_col : bias_col + 1], scale=scale,
                )

            # per-slot phase-split activation buffers
            bufEt = {}

            def phase_evict(bEO, half, ts_, cols, ps, bias_col, scale=1.0):
                # one ACT: in [96, 2, cols/2] (phase, t) <- psum pairs;
                # out col = phase*half + ts_/2 + t
                h = cols // 2
                inv = ps[0:96, 0:cols].rearrange("p (t q) -> p q t", q=2)
                outv = bEO[0:96, 0 : 2 * half].rearrange(
                    "p (q t) -> p q t", q=2)[:, :, ts_ // 2 : ts_ // 2 + h]
                act(outv, inv, bias_col, scale=scale)

            gdir = {}

            def gseq(key, n):
                d = gdir.get(key, 0)
                gdir[key] = 1 - d
                return range(n - 1, -1, -1) if d else range(n)

            def emit_l1(j):
                """fp8 DoubleRow hi/lo L1: 8 DR matmuls per psum tile."""
                x2 = x2t[j]
                th = T0[j] // 2
                xv = x2[0:80, :].rearrange("p (q t) -> p q t", q=3)
                t1 = T1[j]
                bEO = ap_.tile([96, t1], F32R, tag="b1", name=f"b1_{j}")
                half = t1 // 2
                sclv = bs[0:96, 10:11]
                for ts_ in range(0, t1, 2048):
                    cols = min(2048, t1 - ts_)
                    ps = pp.tile([96, 2048], F32, tag="conv", name=f"ps1_{j}_{ts_}")
                    for gi, g in enumerate(gseq("l1", 8)):
                        lhsT = w1s[0:80, 192 * g : 192 * (g + 1)].rearrange(
                            "p (q m) -> p q m", q=2)
                        if g < 5:
                            qsl = (0, 2, 1)  # slabs (xh, xl), stride th
                            d = g
                        else:
                            qsl = (0, 3, 2)  # slabs (xh, xh<<1), stride 2*th
                            d = 2 * (g - 5)
                        for cs in range(0, cols, 512):
                            cn = min(512, cols - cs)
                            t_off = ts_ + cs + d
                            rhs = xv[:, qsl[0] : qsl[1] : qsl[2],
                                     t_off : t_off + cn]
                            nc.tensor.matmul(
                                ps[0:96, cs : cs + cn],
                                lhsT,
                                rhs,
                                start=(gi == 0),
                                stop=(gi == 7),
                                perf_mode=DR,
                            )
                    phase_evict(bEO, half, ts_, cols, ps, 0, scale=sclv)
                bufEt[(1, j)] = bEO

            # tap order for K=5 on phase-split input: (E,0),(O,0),(E,1),(O,1),(E,2)
            PHASES5 = ((0, 0), (1, 0), (0, 1), (1, 1), (0, 2))

            def emit_conv(j, lsrc, ldst, w_sb, tin_all, tout_all, bias_col,
                          out_dtype=F32R):
                tout = tout_all[j]
                srcEO = bufEt[(lsrc, j)]
                shalf = tin_all[j] // 2
                bEO = ap_.tile([96, tout], out_dtype, tag=f"b{ldst}",
                               name=f"b{ldst}_{j}")
                half = tout // 2
                for ts_ in range(0, tout, 2048):
                    cols = min(2048, tout - ts_)
                    ps = pp.tile([96, 2048], F32, tag="conv", name=f"psc{ldst}_{j}_{ts_}")
                    for gi, g in enumerate(gseq(f"l{ldst}", 5)):
                        ph, d = PHASES5[g]
                        lhsT = w_sb[:, 96 * g : 96 * (g + 1)]
                        off = ph * shalf + d
                        for cs in range(0, cols, 512):
                            cn = min(512, cols - cs)
                            t_off = ts_ + cs
                            nc.tensor.matmul(
                                ps[0:96, cs : cs + cn],
                                lhsT,
                                srcEO[0:96, off + t_off : off + t_off + cn],
                                start=(gi == 0),
                                stop=(gi == 4),
                            )
                    phase_evict(bEO, half, ts_, cols, ps, bias_col)
                bufEt[(ldst, j)] = bEO

            def emit_l4pool(j):
                t4 = T4[j]
                t4p = T4p[j]
                srcEO = bufEt[(3, j)]
                shalf = T3[j] // 2
                w4 = w4bf if t4 < BF4_THRESH else wls[4]
                ps = pp.tile([96, 512], F32, tag="conv", name=f"ps4_{j}")
                for gi, g in enumerate(gseq("l4", 5)):
                    ph, d = PHASES5[g]
                    off = ph * shalf + d
                    nc.tensor.matmul(
                        ps[0:96, 0:t4],
                        w4[:, 96 * g : 96 * (g + 1)],
                        srcEO[0:96, off : off + t4],
                        start=(gi == 0),
                        stop=(gi == 4),
                    )
                # pool pairs are exactly (even, odd) psum columns
                tE = ap_.tile([96, t4p], F32, tag="t4e", name=f"t4e_{j}")
                nc.vector.tensor_copy(tE[:, 0:t4p], ps[0:96, 0 : 2 * t4p - 1 : 2])
                t4s = ap_.tile([96, t4p], F32, tag="t4s", name=f"t4s_{j}")
                nc.vector.tensor_add(
                    t4s[:, 0:t4p],
                    tE[0:96, 0:t4p],
                    ps[0:96, 1 : 2 * t4p : 2],
                )
                # relu(e+o+2*b4), phase-split into slot-interleaved cat buffers
                nE = (t4p + 1) // 2
                nO = t4p // 2
                act(p4Ecat[0:96, j : 8 * (nE - 1) + j + 1 : 8],
                    t4s[0:96, 0 : 2 * nE - 1 : 2], 3)
                if nO:
                    act(p4Ocat[0:96, j : 8 * (nO - 1) + j + 1 : 8],
                        t4s[0:96, 1 : 2 * nO : 2], 3)

            # ---- batched tail layers ----
            ps5h = {}

            def emit_tail_l5(s0):
                p4Ev = p4Ecat[0:96, :].rearrange("p (t s) -> p t s", s=N_SLOTS)
                p4Ov = p4Ocat[0:96, :].rearrange("p (t s) -> p t s", s=N_SLOTS)
                ps5 = pp.tile([96, 4 * T5u], F32, tag="conv", name=f"ps5_{s0}")
                for gi, g in enumerate(gseq("l5", 5)):
                    ph, d = PHASES5[g]
                    src = p4Ov if ph else p4Ev
                    nc.tensor.matmul(
                        ps5[0:96, :],
                        wls[5][:, 96 * g : 96 * (g + 1)],
                        src[:, d : d + T5u, s0 : s0 + 4],
                        start=(gi == 0),
                        stop=(gi == 4),
                    )
                ps5h[s0] = ps5

            def emit_tail_pool5(s0):
                ps5 = ps5h[s0]
                b5v = ps5[0:96, :].rearrange("p (t s) -> p t s", s=4)
                t5e = ap_.tile([96, 4 * T5pu], F32, tag="t5e", name=f"t5e_{s0}")
                nc.vector.tensor_copy(
                    t5e[:].rearrange("p (t s) -> p t s", s=4),
                    b5v[:, 0 : 2 * T5pu : 2, :],
                )
                tmp5 = ap_.tile([96, 4 * T5pu], F32, tag="t5", name=f"t5_{s0}")
                nc.vector.tensor_add(
                    tmp5[:].rearrange("p (t s) -> p t s", s=4),
                    t5e[:].rearrange("p (t s) -> p t s", s=4),
                    b5v[:, 1 : 2 * T5pu : 2, :],
                )
                t5v = tmp5[:].rearrange("p (u s) -> p u s", s=4)
                act(p5Ecat[0:96, :].rearrange("p (u s) -> p u s", s=N_SLOTS)[:, 0:P5E, s0 : s0 + 4],
                    t5v[:, 0 : 2 * P5E - 1 : 2, :], 4)
                act(p5Ocat[0:96, :].rearrange("p (u s) -> p u s", s=N_SLOTS)[:, 0:P5O, s0 : s0 + 4],
                    t5v[:, 1 : 2 * P5O : 2, :], 4)

            def emit_tail_rest():
                p5Ev = p5Ecat[0:96, :]
                p5Ov = p5Ocat[0:96, :]
                ps6 = pp.tile([96, N_SLOTS * T6u], F32, tag="conv")
                for g, (ph, d) in enumerate(PHASES5):
                    src = p5Ov if ph else p5Ev
                    nc.tensor.matmul(
                        ps6[0:96, :],
                        w6bf[:, 96 * g : 96 * (g + 1)],
                        src[:, 8 * d : 8 * (d + T6u)],
                        start=(g == 0),
                        stop=(g == 4),
                    )
                ps6v = ps6[0:96, 0 : 8 * T6u].rearrange("p (t s) -> p t s", s=N_SLOTS)
                act(b6Ecat[0:96, :].rearrange("p (v s) -> p v s", s=N_SLOTS),
                    ps6v[:, 0 : 2 * P6E - 1 : 2, :], 5)
                act(b6Ocat[0:96, :].rearrange("p (v s) -> p v s", s=N_SLOTS),
                    ps6v[:, 1 : 2 * P6O : 2, :], 5)

                ps7 = pp.tile([128, N_SLOTS * T7u], F32, tag="conv")
                for g, (src, d) in enumerate(((b6Ecat, 0), (b6Ocat, 0), (b6Ecat, 1))):
                    nc.tensor.matmul(
                        ps7[0:128, :],
                        w7bf[:, 128 * g : 128 * (g + 1)],
                        src[0:96, 8 * d : 8 * (d + T7u)],
                        start=(g == 0),
                        stop=(g == 2),
                    )
                act(b7cat[:], ps7[0:128, :], 6, prange=128)

            # ---- paired slot emission, smallest capacities first; the
            # slots 4-7 tail work is interleaved into later conv pairs ----
            def emit_pair(pi, a, b, pair_order):
                emit_l1(a)
                emit_l1(b)
                if pi + 1 < len(pair_order):
                    emit_load(pair_order[pi + 1][0])
                    emit_load(pair_order[pi + 1][1])
                emit_conv(a, 1, 2, wls[2], T1, T2, 1)
                emit_conv(b, 1, 2, wls[2], T1, T2, 1)
                dt_a = BF16 if T4[a] < BF4_THRESH else F32R
                dt_b = BF16 if T4[b] < BF4_THRESH else F32R
                emit_conv(a, 2, 3, wls[3], T2, T3, 2, out_dtype=dt_a)
                emit_conv(b, 2, 3, wls[3], T2, T3, 2, out_dtype=dt_b)
                emit_l4pool(a)
                emit_l4pool(b)

            pair_order = [(7, 6), (5, 4), (3, 2), (1, 0)]
            emit_pair(0, 7, 6, pair_order)
            emit_pair(1, 5, 4, pair_order)
            emit_tail_l5(4)
            emit_pair(2, 3, 2, pair_order)
            emit_tail_pool5(4)
            emit_pair(3, 1, 0, pair_order)
            emit_tail_l5(0)
            emit_tail_pool5(0)
            emit_tail_rest()

            # ---- ragged masked max + MLP head ----
            tmpm = ap_.tile([128, N_SLOTS * T7u], F32, tag="tm")
            nc.vector.tensor_add(tmpm[:], b7cat[:], fms[:])
            xmax = ap_.tile([128, N_SLOTS], F32R, tag="xmax")
            nc.vector.reduce_max(
                xmax[:],
                tmpm[:].rearrange("p (t s) -> p s t", s=N_SLOTS),
                axis=AX.X,
            )

            psm1 = pp.tile([128, N_SLOTS], F32, tag="conv")
            nc.tensor.matmul(psm1[0:128, :], lw1s, xmax[:], start=True, stop=True)
            h1 = ap_.tile([128, N_SLOTS], F32R, tag="h1")
            act(h1[:], psm1[0:128, :], 7, prange=128)

            psm2 = pp.tile([64, N_SLOTS], F32, tag="conv")
            nc.tensor.matmul(psm2[0:64, :], lw2s, h1[:], start=True, stop=True)
            h2 = ap_.tile([64, N_SLOTS], F32R, tag="h2")
            act(h2[:], psm2[0:64, :], 8, prange=64)

            psm3 = pp.tile([5, N_SLOTS], F32, tag="conv")
            nc.tensor.matmul(psm3[0:5, :], lw3s, h2[0:64, :], start=True, stop=True)
            outsb = ap_.tile([5, N_SLOTS], F32, tag="osb")
            nc.vector.tensor_scalar_add(outsb[:], psm3[0:5, :], bs[0:5, 9:10])
            nc.sync.dma_start(out_t[:], outsb[:])

    nc.compile()
    return nc


def _prep_x(x, b, cap, sx):
    """Host-side input re-layout: phase-major polyphase fp8 hi/lo slabs
    [x_h | x_l | x_h shifted left one col] -> [80, 3*cap//2]."""
    xb = np.asarray(x[b, :, :cap], np.float32) * sx
    th = cap // 2
    xph = np.concatenate([xb[:, 0 : 2 * th : 2], xb[:, 1 : 2 * th : 2]], axis=0)
    xh = xph.astype(E4NP)
    xl = (xph - xh.astype(np.float32)).astype(E4NP)
    xs1 = np.zeros_like(xh)
    xs1[:, : th - 1] = xh[:, 1:]
    return np.concatenate([xh, xl, xs1], axis=1)


def _w1_blocks(w1s, dtype):
    """[96,40,10] -> polyphase [80, 5, 96] blocks (rows p*40+c, m, o)."""
    return np.ascontiguousarray(
        w1s.transpose(1, 2, 0).reshape(40, 5, 2, 96).transpose(2, 0, 1, 3).reshape(80, 5, 96)
    ).astype(dtype)


def _conv_pack(w, scale=1.0):
    """[O,96,5] -> [96, 5*96] stationary layout."""
    return np.ascontiguousarray(
        np.asarray(w, np.float32).transpose(1, 2, 0).reshape(96, 480) * scale
    )


def _prep_weights(inp, caps):
    """Host-side weight/bias re-layout (all tiny)."""
    poff, PCOLS = _pack_cols(caps)
    T7u = _uniform_tail(caps)[4]

    # fp8 hi/lo L1 stationary: per-out-channel pow2 scale into e4m3 range
    w1 = np.asarray(inp["w1"], np.float32)  # [96, 40, 10]
    x = np.asarray(inp["x_input"], np.float32)
    sx = float(2.0 ** np.floor(np.log2(120.0 / max(np.abs(x).max(), 1e-30))))
    so = 2.0 ** np.floor(
        np.log2(120.0 / (np.abs(w1).max(axis=(1, 2)) + 1e-30))
    ).astype(np.float32)  # [96]
    W1 = w1 * so[:, None, None]
    Bh = _w1_blocks(W1.astype(E4NP).astype(np.float32), E4NP)  # [80,5,96]
    Bl = _w1_blocks(W1 - W1.astype(E4NP).astype(np.float32), E4NP)
    w1dr = np.zeros((80, 8, 2, 96), E4NP)
    for m in range(5):
        w1dr[:, m, 0] = Bh[:, m]
        w1dr[:, m, 1] = Bh[:, m]
    w1dr[:, 5, 0] = Bl[:, 0]
    w1dr[:, 5, 1] = Bl[:, 1]
    w1dr[:, 6, 0] = Bl[:, 2]
    w1dr[:, 6, 1] = Bl[:, 3]
    w1dr[:, 7, 0] = Bl[:, 4]

    wpk = np.zeros((128, PCOLS), np.float32)
    wpk[0:96, poff["w2"] : poff["w2"] + 480] = _conv_pack(inp["w2"])
    wpk[0:96, poff["w3"] : poff["w3"] + 480] = _conv_pack(inp["w3"])
    wpk[0:96, poff["w4"] : poff["w4"] + 480] = _conv_pack(inp["w4"])
    wpk[0:96, poff["w5"] : poff["w5"] + 480] = _conv_pack(inp["w5"], 0.5)
    wpk[0:128, poff["lw1"] : poff["lw1"] + 128] = np.asarray(inp["lw1"], np.float32).T
    wpk[0:128, poff["lw2"] : poff["lw2"] + 64] = np.asarray(inp["lw2"], np.float32).T
    wpk[0:64, poff["lw3"] : poff["lw3"] + 5] = np.asarray(inp["lw3"], np.float32).T

    bc = poff["bias"]
    wpk[0:96, bc + 0] = np.asarray(inp["b1"], np.float32)
    wpk[0:96, bc + 1] = np.asarray(inp["b2"], np.float32)
    wpk[0:96, bc + 2] = np.asarray(inp["b3"], np.float32)
    wpk[0:96, bc + 3] = 2.0 * np.asarray(inp["b4"], np.float32)
    wpk[0:96, bc + 4] = 2.0 * np.asarray(inp["b5"], np.float32)
    wpk[0:96, bc + 5] = np.asarray(inp["b6"], np.float32)
    wpk[0:128, bc + 6] = np.asarray(inp["b7"], np.float32)
    wpk[0:128, bc + 7] = np.asarray(inp["lb1"], np.float32)
    wpk[0:64, bc + 8] = np.asarray(inp["lb2"], np.float32)
    wpk[0:5, bc + 9] = np.asarray(inp["lb3"], np.float32)
    wpk[0:96, bc + 10] = 1.0 / (sx * so)  # L1 eviction dequant scale

    wbf = np.zeros((96, 1344), BFNP)
    wbf[:, 0:480] = _conv_pack(inp["w4"]).astype(BFNP)
    wbf[:, 480:960] = _conv_pack(inp["w6"], 0.5).astype(BFNP)
    w7 = np.asarray(inp["w7"], np.float32)  # [128, 96, 3]
    wbf[:, 960:1344] = np.ascontiguousarray(
        w7.transpose(1, 2, 0).reshape(96, 384)
    ).astype(BFNP)

    return {"w1dr": w1dr.reshape(80, 8 * 192), "wbf": wbf}, wpk, sx


def _schedule(len_mask):
    """Sort samples by length desc, deal round-robin: core c, slot j gets
    sample order[8j + c].  Slot capacity = rank-group max."""
    lens = np.asarray(len_mask, np.int64).clip(1, T_FULL)
    order = np.argsort(-lens, kind="stable")
    sample_of = np.zeros((N_CORES, N_SLOTS), np.int64)
    caps = []
    for j in range(N_SLOTS):
        grp = order[j * N_CORES : (j + 1) * N_CORES]
        for c in range(N_CORES):
            sample_of[c, j] = grp[c]
        cap = int(lens[grp].max())
        cap = max(cap, 1312)  # keep the whole chain >= 1 frame
        # round up to a multiple of 32 so T1..T4 are all even
        # (fp32r matmuls require an even moving-operand size)
        cap = min(((cap + 31) // 32) * 32, T_FULL)
        caps.append(cap)
    return order, sample_of, caps


def _make_in_maps(inputs, sample_of, caps):
    x = np.asarray(inputs["x_input"], np.float32)
    len_mask = np.asarray(inputs["len_mask"], np.int32)
    T7u = _uniform_tail(caps)[4]
    w, wpk_base, sx = _prep_weights(inputs, caps)
    poff, _ = _pack_cols(caps)
    in_maps = []
    for c in range(N_CORES):
        m = dict(w)
        wpk = wpk_base.copy()
        # slot-interleaved mask layout: column = t*8 + s
        fm2 = np.full((T7u, N_SLOTS), NEG, np.float32)
        for j in range(N_SLOTS):
            bidx = int(sample_of[c, j])
            m[f"x{j}"] = _prep_x(x, bidx, caps[j], sx)
            lv7 = _chain(int(max(min(len_mask[bidx], T_FULL), 1312)))[8]
            lv7 = max(min(lv7, T7u), 1)
            fm2[0:lv7, j] = 0.0
        wpk[:, poff["fmask"] : poff["fmask"] + N_SLOTS * T7u] = np.broadcast_to(
            fm2.reshape(-1)[None, :], (128, N_SLOTS * T7u)
        )
        m["wpk"] = wpk
        in_maps.append(m)
    return in_maps


def _ensure_ntff_hook():
    """The agent image lacks ``antenv.axon_hooks``; seed a shim so
    ``run_bass_kernel_spmd(trace=True)`` can reach the axon NTFF profiler."""
    import types

    if "antenv.axon_hooks" in sys.modules:
        return
    try:
        from trn_agent_boot.trn_boot import _ntff_profile_via_ctypes

        hook = _ntff_profile_via_ctypes("/opt/axon/libaxon_pjrt.so")
    except Exception:
        hook = None
    mod = types.ModuleType("antenv.axon_hooks")
    state = {"hook": hook}
    mod.get_axon_ntff_profile_hook = lambda: state["hook"]
    mod.set_axon_ntff_profile_hook = lambda h: state.update(hook=h)
    sys.modules["antenv.axon_hooks"] = mod


_LDW_PATCHED = False


def _enable_ldw_opt():
    """Turn on walrus's LDWEIGHTS dedup (drops redundant weight reloads for
    back-to-back same-weight matmuls).  Verified bit-identical results."""
    global _LDW_PATCHED
    if _LDW_PATCHED:
        return
    try:
        import concourse.bass_utils as bu

        _orig = bu.run_command

        def run_command_ldw(argv, **kw):
            argv = [
                "--enable-ldw-opt=true" if a == "--enable-ldw-opt=false" else a
                for a in argv
            ]
            return _orig(argv, **kw)

        bu.run_command = run_command_ldw
        _LDW_PATCHED = True
    except Exception:
        pass


def _run(inputs, trace=False):
    if trace:
        _ensure_ntff_hook()
    # ldw-opt (LDWEIGHTS dedup) is incompatible with DoubleRow matmuls;
    # chunk matmuls are long enough to hide the per-chunk reloads.
    len_mask = np.asarray(inputs["len_mask"], np.int32)
    order, sample_of, caps = _schedule(len_mask)
    nc = _build_program(caps)
    in_maps = _make_in_maps(inputs, sample_of, caps)
    res = run_bass_kernel_spmd(
        nc, in_maps, core_ids=list(range(N_CORES)), trace=trace
    )
    out = np.zeros((B, 5), np.float32)
    for c in range(N_CORES):
        o = res.results[c]["out"]  # [5, 8]
        for j in range(N_SLOTS):
            out[int(sample_of[c, j])] = o[:, j]
    return out, res


def kernel(**inputs):
    out, _ = _run(inputs, trace=False)
    return out


# revision 14
# speedup vs baseline: 1.3963x; 1.3963x over previous
"""Trainium2 Bass kernel for the conv1d-stack + MLP ragged-sequence model.

Strategy:
  - Pure data parallelism over 8 NeuronCores: 8 samples per core.
  - Samples are sorted by valid length (descending) and dealt round-robin to
    cores, so every core runs an IDENTICAL (SPMD) program whose per-slot
    sizes are the per-rank-group maximum length (exploits raggedness).
  - Convs run as polyphase matmuls on phase-split activations (even/odd
    time samples separated), turning every stride-2 conv into stride-1
    matmul reads at 1 col/cycle.  L1 inputs/weights ship in bf16 (same PE
    rate as fp32r, half the HBM traffic); L2-L5 stay fp32r.
  - L6, L7 and small-slot L4 run in bf16: fp32r moving operands under 256
    columns drop to 1/4 rate on a warm PE, bf16 keeps 1 col/cycle.
  - Layers 5-7 interleave all 8 slots (column = t*8 + s) so their matmul
    reads are single contiguous slabs.
  - Slots are emitted in pairs so PSUM-eviction latency of slot j hides
    behind slot j+1's matmuls; the slot 4-7 tail (L5/pool) is emitted in
    the middle of the remaining conv pairs to shorten the serial tail.
  - All f32 weights/biases/masks ship as ONE packed DRAM tensor (single
    DMA issue), fp8 and bf16 weights as one pack each: startup is gated by
    two small x-slab DMAs instead of a dozen serialized issues.
  - The ragged masked-max uses a host-built additive -1e30 mask (a data
    input, SPMD-safe), then the 3-layer MLP runs on-device.
"""

import os
import sys

for _p in ("/opt/trn_rl_repo",):
    if _p not in sys.path and os.path.isdir(_p):
        sys.path.insert(0, _p)

import numpy as np
import ml_dtypes

import concourse.bass as bass  # noqa: F401  (registers types)
from concourse import bacc
import concourse.tile as tile
import concourse.mybir as mybir
from concourse.bass_utils import run_bass_kernel_spmd

F32 = mybir.dt.float32
F32R = mybir.dt.float32r
F8 = mybir.dt.float8e4
BF16 = mybir.dt.bfloat16
BFNP = ml_dtypes.bfloat16
AF = mybir.ActivationFunctionType
AX = mybir.AxisListType

N_CORES = 8
N_SLOTS = 8
B = 64
C_IN = 40
T_FULL = 8192
NEG = -1.0e30
BF4_THRESH = 256  # L4 slots with fewer moving cols than this run in bf16


def _chain(t0):
    """Per-layer valid/capacity length chain (mirrors the reference)."""
    t1 = (t0 - 10) // 2 + 1
    t2 = (t1 - 5) // 2 + 1
    t3 = (t2 - 5) // 2 + 1
    t4 = (t3 - 5) // 2 + 1
    t4p = t4 // 2
    t5 = (t4p - 5) // 2 + 1
    t5p = t5 // 2
    t6 = (t5p - 5) // 2 + 1
    t7 = (t6 - 3) // 2 + 1
    return t1, t2, t3, t4, t4p, t5, t5p, t6, t7


def _uniform_tail(caps):
    T4P0 = _chain(caps[0])[4]
    T5u = (T4P0 - 5) // 2 + 1
    T5pu = T5u // 2
    T6u = (T5pu - 5) // 2 + 1
    T7u = (T6u - 3) // 2 + 1
    return T4P0, T5u, T5pu, T6u, T7u


def _pack_cols(caps):
    """Column layout of the packed f32 weight tensor."""
    T7u = _uniform_tail(caps)[4]
    off = {}
    c = 0
    for name, w in (("w1", 480), ("w2", 480), ("w3", 480), ("w4", 480),
                    ("w5", 480), ("w6", 480), ("w7", 384),
                    ("lw1", 128), ("lw2", 64), ("lw3", 5), ("bias", 11),
                    ("fmask", N_SLOTS * T7u)):
        off[name] = c
        c += w
    return off, c


def _build_program(caps):
    """Emit the SPMD Bass/Tile program for the given 8 slot capacities."""
    T0 = list(caps)
    T1, T2, T3, T4, T4p = [], [], [], [], []
    for t0 in T0:
        t1, t2, t3, t4, t4p, _, _, _, _ = _chain(t0)
        T1.append(t1)
        T2.append(t2)
        T3.append(t3)
        T4.append(t4)
        T4p.append(t4p)
    T4P0, T5u, T5pu, T6u, T7u = _uniform_tail(caps)
    P4E = (T4P0 + 1) // 2
    P4O = T4P0 // 2
    P5E = (T5pu + 1) // 2
    P5O = T5pu // 2
    P6E = (T6u + 1) // 2
    P6O = T6u // 2
    poff, PCOLS = _pack_cols(caps)

    nc = bacc.Bacc("TRN2", target_bir_lowering=False, debug=False)

    xs = [
        nc.dram_tensor(f"x{j}", [80, T0[j] // 2], F32R, kind="ExternalInput")
        for j in range(N_SLOTS)
    ]
    wpk_t = nc.dram_tensor("wpk", [128, PCOLS], F32R, kind="ExternalInput")
    out_t = nc.dram_tensor("out", [5, N_SLOTS], F32, kind="ExternalOutput")

    with tile.TileContext(nc) as tc:
        with (
            tc.tile_pool(name="wp", bufs=1) as wp,
            tc.tile_pool(name="xp", bufs=3) as xp,
            tc.tile_pool(name="actp", bufs=2) as ap_,
            tc.tile_pool(name="catp", bufs=1) as cp,
            tc.tile_pool(name="psp", bufs=2, space="PSUM") as pp,
        ):
            # ---- PE warm-up tile memset first so the PE can ramp while the
            # first x slabs stream in (outputs are never read) ----
            wtile = ap_.tile([80, 512], F32R, tag="wtile")
            nc.gpsimd.memset(wtile[:].bitcast(F32), 0.0)

            # ---- slot-0/1 inputs first (they gate the first matmuls) ----
            x2t = [None] * N_SLOTS
            _ldq = [nc.sync, nc.gpsimd]
            _ldn = [0]

            def emit_load(j):
                th = T0[j] // 2
                x2 = xp.tile([80, th], F32R, tag="x2", name=f"x2_{j}")
                cuts = [0, 2056, th] if th > 2100 else [0, th]
                for ci in range(len(cuts) - 1):
                    lo, hi = cuts[ci], cuts[ci + 1]
                    q = _ldq[_ldn[0] % 2]
                    _ldn[0] += 1
                    q.dma_start(x2[:, lo:hi], xs[j][:, lo:hi])
                x2t[j] = x2

            emit_load(7)
            emit_load(6)

            # ---- resident weights / constants (3 packed DMAs) ----
            wpk = wp.tile([128, PCOLS], F32R, tag="wpk")
            nc.scalar.dma_start(wpk[:], wpk_t[:])
            wls = {
                l: wpk[0:96, poff[f"w{l}"] : poff[f"w{l}"] + 480]
                for l in (2, 3, 4, 5, 6)
            }
            w1s = wpk[0:80, poff["w1"] : poff["w1"] + 480]
            w7s = wpk[0:96, poff["w7"] : poff["w7"] + 384]
            lw1s = wpk[0:128, poff["lw1"] : poff["lw1"] + 128]
            lw2s = wpk[0:128, poff["lw2"] : poff["lw2"] + 64]
            lw3s = wpk[0:64, poff["lw3"] : poff["lw3"] + 5]
            bs = wpk[0:128, poff["bias"] : poff["bias"] + 11].bitcast(F32)
            fms = wpk[0:128, poff["fmask"] : poff["fmask"] + N_SLOTS * T7u].bitcast(F32)

            # ---- PE warm-up ----
            ps_w = pp.tile([96, 480], F32, tag="conv")
            for wi in range(6):
                nc.tensor.matmul(
                    ps_w[0:96, 0:480],
                    wtile[0:80, 0:96],
                    wtile[0:80, 0:480],
                    start=True,
                    stop=True,
                )

            # ---- concatenated slot-interleaved tail buffers (phase-split) ----
            p4Ecat = cp.tile([96, N_SLOTS * P4E], F32R, tag="p4Ecat")
            p4Ocat = cp.tile([96, N_SLOTS * P4O], F32R, tag="p4Ocat")
            nc.gpsimd.memset(p4Ecat[:].bitcast(F32), 0.0)
            nc.gpsimd.memset(p4Ocat[:].bitcast(F32), 0.0)
            W5E = max(N_SLOTS * P5E, 8 * 2 + 256)
            W5O = max(N_SLOTS * P5O, 8 * 1 + 256)
            W6E = max(N_SLOTS * P6E, 8 * 1 + 256)
            W6O = max(N_SLOTS * P6O, 256)
            p5Ecat = cp.tile([96, W5E], F32R, tag="p5Ecat")
            p5Ocat = cp.tile([96, W5O], F32R, tag="p5Ocat")
            b6Ecat = cp.tile([96, W6E], F32R, tag="b6Ecat")
            b6Ocat = cp.tile([96, W6O], F32R, tag="b6Ocat")
            for _t, _w, _n in ((p5Ecat, W5E, N_SLOTS * P5E),
                               (p5Ocat, W5O, N_SLOTS * P5O),
                               (b6Ecat, W6E, N_SLOTS * P6E),
                               (b6Ocat, W6O, N_SLOTS * P6O)):
                if _w > _n:
                    nc.gpsimd.memset(_t[:, _n:_w].bitcast(F32), 0.0)
            b7cat = cp.tile([128, N_SLOTS * T7u], F32, tag="b7cat")

            def act(dst_ap, src_ap, bias_col, func=AF.Relu, prange=96, scale=1.0):
                nc.scalar.activation(
                    dst_ap, src_ap, func,
                    bias=bs[0:prange, bias_col : bias_col + 1], scale=scale,
                )

            # per-slot phase-split activation buffers
            bufEt = {}

            def phase_evict(bEO, half, ts_, cols, ps, bias_col, scale=1.0):
                # one ACT: in [96, 2, cols/2] (phase, t) <- psum pairs;
                # out col = phase*half + ts_/2 + t
                h = cols // 2
                inv = ps[0:96, 0:cols].rearrange("p (t q) -> p q t", q=2)
                outv = bEO[0:96, 0 : 2 * half].rearrange(
                    "p (q t) -> p q t", q=2)[:, :, ts_ // 2 : ts_ // 2 + h]
                act(outv, inv, bias_col, scale=scale)

            gdir = {}

            def gseq(key, n):
                d = gdir.get(key, 0)
                gdir[key] = 1 - d
                return range(n - 1, -1, -1) if d else range(n)

            def emit_l1(j):
                x2 = x2t[j]
                t1 = T1[j]
                bEO = ap_.tile([96, t1], F32R, tag="b1", name=f"b1_{j}")
                half = t1 // 2
                for ts_ in range(0, t1, 2048):
                    cols = min(2048, t1 - ts_)
                    ps = pp.tile([96, 2048], F32, tag="conv", name=f"ps1_{j}_{ts_}")
                    for gi, g in enumerate(gseq("l1", 5)):
                        lhsT = w1s[:, 96 * g : 96 * (g + 1)]
                        for cs in range(0, cols, 512):
                            cn = min(512, cols - cs)
                            t_off = ts_ + cs
                            nc.tensor.matmul(
                                ps[0:96, cs : cs + cn],
                                lhsT,
                                x2[0:80, t_off + g : t_off + g + cn],
                                start=(gi == 0),
                                stop=(gi == 4),
                            )
                    phase_evict(bEO, half, ts_, cols, ps, 0)
                bufEt[(1, j)] = bEO

            # tap order for K=5 on phase-split input: (E,0),(O,0),(E,1),(O,1),(E,2)
            PHASES5 = ((0, 0), (1, 0), (0, 1), (1, 1), (0, 2))

            def emit_conv(j, lsrc, ldst, w_sb, tin_all, tout_all, bias_col,
                          out_pad=0):
                tout = tout_all[j]
                srcEO = bufEt[(lsrc, j)]
                shalf = tin_all[j] // 2
                bEO = ap_.tile([96, tout + out_pad], F32R, tag=f"b{ldst}",
                               name=f"b{ldst}_{j}")
                if out_pad:
                    nc.gpsimd.memset(bEO[:, tout : tout + out_pad].bitcast(F32), 0.0)
                half = tout // 2
                for ts_ in range(0, tout, 2048):
                    cols = min(2048, tout - ts_)
                    ps = pp.tile([96, 2048], F32, tag="conv", name=f"psc{ldst}_{j}_{ts_}")
                    for gi, g in enumerate(gseq(f"l{ldst}", 5)):
                        ph, d = PHASES5[g]
                        lhsT = w_sb[:, 96 * g : 96 * (g + 1)]
                        off = ph * shalf + d
                        for cs in range(0, cols, 512):
                            cn = min(512, cols - cs)
                            t_off = ts_ + cs
                            nc.tensor.matmul(
                                ps[0:96, cs : cs + cn],
                                lhsT,
                                srcEO[0:96, off + t_off : off + t_off + cn],
                                start=(gi == 0),
                                stop=(gi == 4),
                            )
                    phase_evict(bEO, half, ts_, cols, ps, bias_col)
                bufEt[(ldst, j)] = bEO

            def emit_l4pool(j):
                t4 = T4[j]
                t4p = T4p[j]
                srcEO = bufEt[(3, j)]
                shalf = T3[j] // 2
                n4 = max(t4, 256)  # fp32r below 256 moving cols runs at 1/4 rate
                ps = pp.tile([96, 512], F32, tag="conv", name=f"ps4_{j}")
                for gi, g in enumerate(gseq("l4", 5)):
                    ph, d = PHASES5[g]
                    off = ph * shalf + d
                    nc.tensor.matmul(
                        ps[0:96, 0:n4],
                        wls[4][:, 96 * g : 96 * (g + 1)],
                        srcEO[0:96, off : off + n4],
                        start=(gi == 0),
                        stop=(gi == 4),
                    )
                # pool pairs are exactly (even, odd) psum columns
                tE = ap_.tile([96, t4p], F32, tag="t4e", name=f"t4e_{j}")
                nc.vector.tensor_copy(tE[:, 0:t4p], ps[0:96, 0 : 2 * t4p - 1 : 2])
                t4s = ap_.tile([96, t4p], F32, tag="t4s", name=f"t4s_{j}")
                nc.vector.tensor_add(
                    t4s[:, 0:t4p],
                    tE[0:96, 0:t4p],
                    ps[0:96, 1 : 2 * t4p : 2],
                )
                # relu(e+o+2*b4), phase-split into slot-interleaved cat buffers
                nE = (t4p + 1) // 2
                nO = t4p // 2
                act(p4Ecat[0:96, j : 8 * (nE - 1) + j + 1 : 8],
                    t4s[0:96, 0 : 2 * nE - 1 : 2], 3)
                if nO:
                    act(p4Ocat[0:96, j : 8 * (nO - 1) + j + 1 : 8],
                        t4s[0:96, 1 : 2 * nO : 2], 3)

            # ---- batched tail layers ----
            ps5h = {}

            def emit_tail_l5(s0):
                p4Ev = p4Ecat[0:96, :].rearrange("p (t s) -> p t s", s=N_SLOTS)
                p4Ov = p4Ocat[0:96, :].rearrange("p (t s) -> p t s", s=N_SLOTS)
                ps5 = pp.tile([96, 4 * T5u], F32, tag="conv", name=f"ps5_{s0}")
                for gi, g in enumerate(gseq("l5", 5)):
                    ph, d = PHASES5[g]
                    src = p4Ov if ph else p4Ev
                    nc.tensor.matmul(
                        ps5[0:96, :],
                        wls[5][:, 96 * g : 96 * (g + 1)],
                        src[:, d : d + T5u, s0 : s0 + 4],
                        start=(gi == 0),
                        stop=(gi == 4),
                    )
                ps5h[s0] = ps5

            def emit_tail_pool5(s0):
                ps5 = ps5h[s0]
                b5v = ps5[0:96, :].rearrange("p (t s) -> p t s", s=4)
                t5e = ap_.tile([96, 4 * T5pu], F32, tag="t5e", name=f"t5e_{s0}")
                nc.vector.tensor_copy(
                    t5e[:].rearrange("p (t s) -> p t s", s=4),
                    b5v[:, 0 : 2 * T5pu : 2, :],
                )
                tmp5 = ap_.tile([96, 4 * T5pu], F32, tag="t5", name=f"t5_{s0}")
                nc.vector.tensor_add(
                    tmp5[:].rearrange("p (t s) -> p t s", s=4),
                    t5e[:].rearrange("p (t s) -> p t s", s=4),
                    b5v[:, 1 : 2 * T5pu : 2, :],
                )
                t5v = tmp5[:].rearrange("p (u s) -> p u s", s=4)
                act(p5Ecat[0:96, :].rearrange("p (u s) -> p u s", s=N_SLOTS)[:, 0:P5E, s0 : s0 + 4],
                    t5v[:, 0 : 2 * P5E - 1 : 2, :], 4)
                act(p5Ocat[0:96, :].rearrange("p (u s) -> p u s", s=N_SLOTS)[:, 0:P5O, s0 : s0 + 4],
                    t5v[:, 1 : 2 * P5O : 2, :], 4)

            def emit_tail_rest():
                p5Ev = p5Ecat[0:96, :]
                p5Ov = p5Ocat[0:96, :]
                n6 = max(N_SLOTS * T6u, 256)
                ps6 = pp.tile([96, 512], F32, tag="conv")
                for g, (ph, d) in enumerate(PHASES5):
                    src = p5Ov if ph else p5Ev
                    nc.tensor.matmul(
                        ps6[0:96, 0:n6],
                        wls[6][:, 96 * g : 96 * (g + 1)],
                        src[:, 8 * d : 8 * d + n6],
                        start=(g == 0),
                        stop=(g == 4),
                    )
                ps6v = ps6[0:96, 0 : 8 * T6u].rearrange("p (t s) -> p t s", s=N_SLOTS)
                act(b6Ecat[0:96, 0 : N_SLOTS * P6E].rearrange("p (v s) -> p v s", s=N_SLOTS),
                    ps6v[:, 0 : 2 * P6E - 1 : 2, :], 5)
                act(b6Ocat[0:96, 0 : N_SLOTS * P6O].rearrange("p (v s) -> p v s", s=N_SLOTS),
                    ps6v[:, 1 : 2 * P6O : 2, :], 5)

                n7 = max(N_SLOTS * T7u, 256)
                ps7 = pp.tile([128, 512], F32, tag="conv")
                for g, (src, d) in enumerate(((b6Ecat, 0), (b6Ocat, 0), (b6Ecat, 1))):
                    nc.tensor.matmul(
                        ps7[0:128, 0:n7],
                        w7s[:, 128 * g : 128 * (g + 1)],
                        src[0:96, 8 * d : 8 * d + n7],
                        start=(g == 0),
                        stop=(g == 2),
                    )
                act(b7cat[:], ps7[0:128, 0 : N_SLOTS * T7u], 6, prange=128)

            # ---- paired slot emission, smallest capacities first; the
            # slots 4-7 tail work is interleaved into later conv pairs ----
            def emit_pair(pi, a, b, pair_order):
                emit_l1(a)
                emit_l1(b)
                if pi + 1 < len(pair_order):
                    emit_load(pair_order[pi + 1][0])
                    emit_load(pair_order[pi + 1][1])
                emit_conv(a, 1, 2, wls[2], T1, T2, 1)
                emit_conv(b, 1, 2, wls[2], T1, T2, 1)
                pad_a = max(0, T3[a] // 2 + 4 + 256 - T3[a]) if T4[a] < 256 else 0
                pad_b = max(0, T3[b] // 2 + 4 + 256 - T3[b]) if T4[b] < 256 else 0
                emit_conv(a, 2, 3, wls[3], T2, T3, 2, out_pad=pad_a)
                emit_conv(b, 2, 3, wls[3], T2, T3, 2, out_pad=pad_b)
                emit_l4pool(a)
                emit_l4pool(b)

            pair_order = [(7, 6), (5, 4), (3, 2), (1, 0)]
            emit_pair(0, 7, 6, pair_order)
            emit_pair(1, 5, 4, pair_order)
            emit_tail_l5(4)
            emit_pair(2, 3, 2, pair_order)
            emit_tail_pool5(4)
            emit_pair(3, 1, 0, pair_order)
            emit_tail_l5(0)
            emit_tail_pool5(0)
            emit_tail_rest()

            # ---- ragged masked max + MLP head ----
            tmpm = ap_.tile([128, N_SLOTS * T7u], F32, tag="tm")
            nc.vector.tensor_add(tmpm[:], b7cat[:], fms[:])
            xmax = ap_.tile([128, N_SLOTS], F32R, tag="xmax")
            nc.vector.reduce_max(
                xmax[:],
                tmpm[:].rearrange("p (t s) -> p s t", s=N_SLOTS),
                axis=AX.X,
            )

            psm1 = pp.tile([128, N_SLOTS], F32, tag="conv")
            nc.tensor.matmul(psm1[0:128, :], lw1s, xmax[:], start=True, stop=True)
            h1 = ap_.tile([128, N_SLOTS], F32R, tag="h1")
            act(h1[:], psm1[0:128, :], 7, prange=128)

            psm2 = pp.tile([64, N_SLOTS], F32, tag="conv")
            nc.tensor.matmul(psm2[0:64, :], lw2s, h1[:], start=True, stop=True)
            h2 = ap_.tile([64, N_SLOTS], F32R, tag="h2")
            act(h2[:], psm2[0:64, :], 8, prange=64)

            psm3 = pp.tile([5, N_SLOTS], F32, tag="conv")
            nc.tensor.matmul(psm3[0:5, :], lw3s, h2[0:64, :], start=True, stop=True)
            outsb = ap_.tile([5, N_SLOTS], F32, tag="osb")
            nc.vector.tensor_scalar_add(outsb[:], psm3[0:5, :], bs[0:5, 9:10])
            nc.sync.dma_start(out_t[:], outsb[:])

    nc.compile()
    return nc


def _prep_x(x, b, cap):
    """Host-side input re-layout: phase-major polyphase [80, cap//2]."""
    xb = np.asarray(x[b, :, :cap], np.float32)
    th = cap // 2
    return np.concatenate([xb[:, 0 : 2 * th : 2], xb[:, 1 : 2 * th : 2]], axis=0)


def _conv_pack(w, scale=1.0):
    """[O,96,5] -> [96, 5*96] stationary layout."""
    return np.ascontiguousarray(
        np.asarray(w, np.float32).transpose(1, 2, 0).reshape(96, 480) * scale
    )


def _prep_weights(inp, caps):
    """Host-side weight/bias re-layout (all tiny)."""
    poff, PCOLS = _pack_cols(caps)

    wpk = np.zeros((128, PCOLS), np.float32)
    w1 = np.asarray(inp["w1"], np.float32)  # [96, 40, 10]
    wpk[0:80, poff["w1"] : poff["w1"] + 480] = np.ascontiguousarray(
        w1.transpose(1, 2, 0).reshape(40, 5, 2, 96).transpose(2, 0, 1, 3).reshape(80, 480)
    )
    wpk[0:96, poff["w2"] : poff["w2"] + 480] = _conv_pack(inp["w2"])
    wpk[0:96, poff["w3"] : poff["w3"] + 480] = _conv_pack(inp["w3"])
    wpk[0:96, poff["w4"] : poff["w4"] + 480] = _conv_pack(inp["w4"])
    wpk[0:96, poff["w5"] : poff["w5"] + 480] = _conv_pack(inp["w5"], 0.5)
    wpk[0:96, poff["w6"] : poff["w6"] + 480] = _conv_pack(inp["w6"], 0.5)
    w7 = np.asarray(inp["w7"], np.float32)  # [128, 96, 3]
    wpk[0:96, poff["w7"] : poff["w7"] + 384] = np.ascontiguousarray(
        w7.transpose(1, 2, 0).reshape(96, 384)
    )
    wpk[0:128, poff["lw1"] : poff["lw1"] + 128] = np.asarray(inp["lw1"], np.float32).T
    wpk[0:128, poff["lw2"] : poff["lw2"] + 64] = np.asarray(inp["lw2"], np.float32).T
    wpk[0:64, poff["lw3"] : poff["lw3"] + 5] = np.asarray(inp["lw3"], np.float32).T

    bc = poff["bias"]
    wpk[0:96, bc + 0] = np.asarray(inp["b1"], np.float32)
    wpk[0:96, bc + 1] = np.asarray(inp["b2"], np.float32)
    wpk[0:96, bc + 2] = np.asarray(inp["b3"], np.float32)
    wpk[0:96, bc + 3] = 2.0 * np.asarray(inp["b4"], np.float32)
    wpk[0:96, bc + 4] = 2.0 * np.asarray(inp["b5"], np.float32)
    wpk[0:96, bc + 5] = np.asarray(inp["b6"], np.float32)
    wpk[0:128, bc + 6] = np.asarray(inp["b7"], np.float32)
    wpk[0:128, bc + 7] = np.asarray(inp["lb1"], np.float32)
    wpk[0:64, bc + 8] = np.asarray(inp["lb2"], np.float32)
    wpk[0:5, bc + 9] = np.asarray(inp["lb3"], np.float32)

    return {}, wpk


def _schedule(len_mask):
    """Sort samples by length desc, deal round-robin: core c, slot j gets
    sample order[8j + c].  Slot capacity = rank-group max."""
    lens = np.asarray(len_mask, np.int64).clip(1, T_FULL)
    order = np.argsort(-lens, kind="stable")
    sample_of = np.zeros((N_CORES, N_SLOTS), np.int64)
    caps = []
    for j in range(N_SLOTS):
        grp = order[j * N_CORES : (j + 1) * N_CORES]
        for c in range(N_CORES):
            sample_of[c, j] = grp[c]
        cap = int(lens[grp].max())
        cap = max(cap, 1312)  # keep the whole chain >= 1 frame
        # round up to a multiple of 32 so T1..T4 are all even
        # (fp32r matmuls require an even moving-operand size)
        cap = min(((cap + 31) // 32) * 32, T_FULL)
        caps.append(cap)
    return order, sample_of, caps


def _make_in_maps(inputs, sample_of, caps):
    x = np.asarray(inputs["x_input"], np.float32)
    len_mask = np.asarray(inputs["len_mask"], np.int32)
    T7u = _uniform_tail(caps)[4]
    w, wpk_base = _prep_weights(inputs, caps)
    poff, _ = _pack_cols(caps)
    in_maps = []
    for c in range(N_CORES):
        m = dict(w)
        wpk = wpk_base.copy()
        # slot-interleaved mask layout: column = t*8 + s
        fm2 = np.full((T7u, N_SLOTS), NEG, np.float32)
        for j in range(N_SLOTS):
            bidx = int(sample_of[c, j])
            m[f"x{j}"] = _prep_x(x, bidx, caps[j])
            lv7 = _chain(int(max(min(len_mask[bidx], T_FULL), 1312)))[8]
            lv7 = max(min(lv7, T7u), 1)
            fm2[0:lv7, j] = 0.0
        wpk[:, poff["fmask"] : poff["fmask"] + N_SLOTS * T7u] = np.broadcast_to(
            fm2.reshape(-1)[None, :], (128, N_SLOTS * T7u)
        )
        m["wpk"] = wpk
        in_maps.append(m)
    return in_maps


def _ensure_ntff_hook():
    """The agent image lacks ``antenv.axon_hooks``; seed a shim so
    ``run_bass_kernel_spmd(trace=True)`` can reach the axon NTFF profiler."""
    import types

    if "antenv.axon_hooks" in sys.modules:
        return
    try:
        from trn_agent_boot.trn_boot import _ntff_profile_via_ctypes

        hook = _ntff_profile_via_ctypes("/opt/axon/libaxon_pjrt.so")
    except Exception:
        hook = None
    mod = types.ModuleType("antenv.axon_hooks")
    state = {"hook": hook}
    mod.get_axon_ntff_profile_hook = lambda: state["hook"]
    mod.set_axon_ntff_profile_hook = lambda h: state.update(hook=h)
    sys.modules["antenv.axon_hooks"] = mod


_LDW_PATCHED = False


def _enable_ldw_opt():
    """Turn on walrus's LDWEIGHTS dedup (drops redundant weight reloads for
    back-to-back same-weight matmuls).  Verified bit-identical results."""
    global _LDW_PATCHED
    if _LDW_PATCHED:
        return
    try:
        import concourse.bass_utils as bu

        _orig = bu.run_command

        def run_command_ldw(argv, **kw):
            argv = [
                "--enable-ldw-opt=true" if a == "--enable-ldw-opt=false" else a
                for a in argv
            ]
            return _orig(argv, **kw)

        bu.run_command = run_command_ldw
        _LDW_PATCHED = True
    except Exception:
        pass


def _run(inputs, trace=False):
    if trace:
        _ensure_ntff_hook()
    _enable_ldw_opt()
    len_mask = np.asarray(inputs["len_mask"], np.int32)
    order, sample_of, caps = _schedule(len_mask)
    nc = _build_program(caps)
    in_maps = _make_in_maps(inputs, sample_of, caps)
    res = run_bass_kernel_spmd(
        nc, in_maps, core_ids=list(range(N_CORES)), trace=trace
    )
    out = np.zeros((B, 5), np.float32)
    for c in range(N_CORES):
        o = res.results[c]["out"]  # [5, 8]
        for j in range(N_SLOTS):
            out[int(sample_of[c, j])] = o[:, j]
    return out, res


def kernel(**inputs):
    out, _ = _run(inputs, trace=False)
    return out
